# revision 15
# baseline (speedup 1.0000x reference)
"""GNN message-passing encoder (GAT-style) on 8 Trainium2 NeuronCores.

Self-contained: hardcodes the problem shapes (N=100000, E=1600000, HN=64, L=3).

Sharding: core c owns dst nodes [c*NL, (c+1)*NL). Edges sorted by
(src_chunk g, dst); dst space split into 128-aligned windows; per (g, window)
the tile count T_gw is the max over cores (shared compile-time schedule).

Per layer, on device (all bf16 tables):
- table build: hlT = fc_w[l]^T h; a_s = W_src hl; a_d = W_dst hl + bias;
  node rows [a_s|hl] -> local_table -> AllGather full_table (256B rows);
  a_d rows transposed into SBUF-resident ad_sb[128, NW, 64].
- per block (4096 edges): dma_gather 4096 src rows (bf16, 1 desc each);
  host-built one-hot mtw [128 dstoff x 4096 e] from HBM; expand matmul
  psx = mtw^T @ ad_sb[:, w, :] per tile -> per-edge a_d; alpha = psx + a_s
  + ea*u (broadcast DVE); LRelu; Exp -> contrib [ex*hl | ex] bf16;
  aggregation matmul per tile accumulates into per-window PSUM
  (mt_ew one-hot built on-device); window flush adds into SBUF acc (f32).
- finalize from SBUF acc: h = num/(den+1e-16) -> hT bf16 (or out rows f32).
No HBM scatter/accumulators; only per-edge gather remains indexed.
"""
import os
import sys

for _p in ("/opt/trn_rl_repo",):
    if _p not in sys.path:
        sys.path.insert(0, _p)

import numpy as np
import ml_dtypes

import concourse.bass as bass
import concourse.mybir as mybir
import concourse.tile as tile
from concourse import bacc
from concourse import library_config as libcfg
from concourse.bass_utils import run_bass_kernel_spmd

F32 = mybir.dt.float32
BF16 = mybir.dt.bfloat16
F8 = mybir.dt.float8e4
I16 = mybir.dt.int16
AX = mybir.AluOpType
AF = mybir.ActivationFunctionType
BF16NP = ml_dtypes.bfloat16
F8NP = ml_dtypes.float8_e4m3fn


class Cfg:
    def __init__(self):
        self.N, self.E, self.C = 100000, 1600000, 8
        self.HN, self.IN_N, self.L, self.B = 64, 3, 3, 10
        self.NEG, self.EPS = 0.2, 1e-5
        self.NL = self.N // self.C          # 12500
        self.CHUNK = 25000
        self.G = self.N // self.CHUNK       # 4
        self.TE = 128
        self.BT = 32
        self.EB = self.TE * self.BT         # 4096
        self.NW = -(-self.NL // 128)        # 98
        self.NLP = self.NW * 128            # 12544


# ------------------------------------------------------------- host prep ----
def _pack_idx16(vals):
    v = np.asarray(vals, dtype=np.int16)
    assert v.size % 16 == 0
    return np.tile(v.reshape(-1, 16).T, (8, 1)).copy()


def host_prepare(cfg, inputs):
    C, G, NL, CHUNK, NW = cfg.C, cfg.G, cfg.NL, cfg.CHUNK, cfg.NW
    TE, BT, EB, HN, L = cfg.TE, cfg.BT, cfg.EB, cfg.HN, cfg.L

    ei = np.asarray(inputs["edge_index"]).astype(np.int64)
    src, dst = ei[0], ei[1]
    ea_all = np.asarray(inputs["edge_attr"], np.float32)[:, 0]
    core = dst // NL
    dstl = dst - core * NL
    gch = src // CHUNK
    win = dstl >> 7
    dof = dstl & 127

    # shared tile schedule: T_gw = max over cores of ceil(cnt/128)
    key = (core * G + gch) * NW + win
    cnt = np.bincount(key, minlength=C * G * NW).reshape(C, G, NW)
    Tgw = -(-cnt.max(axis=0) // TE)                    # [G, NW]
    tiles_g = Tgw.sum(axis=1)                          # [G]
    nbg = np.maximum(1, -(-tiles_g // BT))
    NB = int(nbg.sum())
    tile_start = np.cumsum(Tgw, axis=1) - Tgw          # [G, NW]
    block_base = np.concatenate([[0], np.cumsum(nbg)])[:G]

    # per-block window schedule (window id per tile slot, -1 = padding)
    wsched = -np.ones((NB, BT), np.int64)
    for g in range(G):
        flat = np.repeat(np.arange(NW), Tgw[g])
        b = block_base[g] + np.arange(len(flat)) // BT
        wsched[b, np.arange(len(flat)) % BT] = flat
    gob = np.repeat(np.arange(G), nbg)

    # weights
    fnw = np.asarray(inputs["fc_node_w"], np.float32)
    fnb = np.asarray(inputs["fc_node_b"], np.float32)
    few = np.asarray(inputs["fc_edge_w"], np.float32)
    feb = np.asarray(inputs["fc_edge_b"], np.float32)
    gam = np.asarray(inputs["bn_gamma"], np.float32)
    bet = np.asarray(inputs["bn_beta"], np.float32)
    fcw = np.asarray(inputs["fc_w"], np.float32)
    fcb = np.asarray(inputs["fc_b"], np.float32)
    aw = np.asarray(inputs["attn_w"], np.float32)
    ab = np.asarray(inputs["attn_b"], np.float32)
    wdst, wsrc, we = aw[:, :HN, :], aw[:, HN:2 * HN, :], aw[:, 2 * HN:, :]
    u = np.stack([few[0] @ we[l] for l in range(L)])          # [L, 64]
    adb = np.stack([ab[l] + feb @ we[l] for l in range(L)])   # [L, 64]
    x = np.asarray(inputs["x"], np.float32)

    shared = dict(
        fnw=fnw, fnb=fnb.reshape(-1, 1), gam=gam.reshape(-1, 1),
        bet=bet.reshape(-1, 1),
        fcw=np.ascontiguousarray(fcw.transpose(1, 0, 2)).astype(BF16NP),
        wsrc=np.ascontiguousarray(wsrc.transpose(1, 0, 2)).astype(BF16NP),
        wdst=np.ascontiguousarray(wdst.transpose(1, 0, 2)).astype(BF16NP),
        fcb=np.ascontiguousarray(fcb.T), adb=np.ascontiguousarray(adb.T),
        u_row=u.reshape(1, -1).astype(BF16NP),
        iota_mod=np.tile(np.arange(128, dtype=BF16NP), (128, BT)),
        identb=np.eye(128, dtype=BF16NP),
        identf=np.eye(128, dtype=np.float32),
    )

    in_maps = []
    for c in range(C):
        m = core == c
        sc, gc, wc, dfc, eac, dlc = (src[m], gch[m], win[m], dof[m],
                                     ea_all[m], dstl[m])
        order = np.lexsort((dlc, gc))
        sc, gc, wc, dfc, eac = (sc[order], gc[order], wc[order], dfc[order],
                                eac[order])
        gw = gc * NW + wc
        cnts = np.bincount(gw, minlength=G * NW)
        starts = np.cumsum(cnts) - cnts
        k = np.arange(sc.size) - starts[gw]
        tin = k >> 7
        p = k & 127
        tgi = tile_start[gc, wc] + tin
        blk = block_base[gc] + tgi // BT
        tib = tgi % BT
        slot = blk * EB + tib * TE + p

        gidx = np.zeros(NB * EB, np.int16)
        gidx[slot] = (sc - gc * CHUNK).astype(np.int16)
        mtw = np.zeros((NB, 128, EB), F8NP)
        mtw[blk, dfc, tib * TE + p] = 1.0
        dsto = np.full((128, NB * BT), 255, BF16NP)
        dsto[p, blk * BT + tib] = dfc.astype(BF16NP)
        eab = np.zeros((NB, EB), BF16NP)
        eab[blk, tib * TE + p] = eac.astype(BF16NP)

        im = dict(shared)
        im.update(gidx=_pack_idx16(gidx), mtw=mtw, dsto=dsto, eab=eab,
                  xT=np.ascontiguousarray(x[c * NL:(c + 1) * NL].T))
        in_maps.append(im)
    return in_maps, wsched, gob, nbg


# --------------------------------------------------------------- builder ----
def split_sync_waits(nc, max_waits=1):
    for f in nc.m.functions:
        for bb in f.blocks:
            old = bb.instructions
            if not any(i.sync_info and i.sync_info.on_wait
                       and len(i.sync_info.on_wait) > max_waits for i in old):
                continue
            new = []
            for ins in old:
                si = ins.sync_info
                if si is not None and si.on_wait and len(si.on_wait) > max_waits:
                    waits = list(si.on_wait)
                    extra, keep = waits[:-max_waits], waits[-max_waits:]
                    for j, w in enumerate(extra):
                        nop = mybir.InstNoOp(name=f"{ins.name}-wc{j}", ins=[], outs=[])
                        nop.engine = ins.engine
                        nop.sync_info = mybir.SyncInfo(on_wait=[w], on_update=[])
                        new.append(nop)
                    si.on_wait = keep
                new.append(ins)
            bb.instructions = new
    return nc


def build_nc(cfg, wsched, gob, nbg):
    NL, G, CHUNK, TE, BT, EB = cfg.NL, cfg.G, cfg.CHUNK, cfg.TE, cfg.BT, cfg.EB
    HN, L, C, N, NW, NLP = cfg.HN, cfg.L, cfg.C, cfg.N, cfg.NW, cfg.NLP
    NB = int(sum(nbg))
    RG = [list(range(C))]
    # live tile count per block (live tiles form a prefix)
    nlive = [(wsched[b] >= 0).sum() for b in range(NB)]

    nc = bacc.Bacc(None, target_bir_lowering=False, num_swdge_queues=4)
    din = {}

    def ext(name, shape, dt=F32):
        din[name] = nc.dram_tensor(name, shape, dt, kind="ExternalInput")

    ext("xT", [cfg.IN_N, NL])
    ext("gidx", [128, NB * EB // 16], I16)
    ext("mtw", [NB, 128, EB], F8)
    ext("dsto", [128, NB * BT], BF16)
    ext("eab", [NB, EB], BF16)
    ext("iota_mod", [128, BT * 128], BF16)
    ext("fnw", [cfg.IN_N, HN]); ext("fnb", [HN, 1])
    ext("gam", [HN, 1]); ext("bet", [HN, 1])
    ext("fcw", [HN, L, HN], BF16); ext("fcb", [HN, L])
    ext("wsrc", [HN, L, HN], BF16); ext("wdst", [HN, L, HN], BF16)
    ext("adb", [HN, L])
    ext("u_row", [1, L * HN], BF16)
    ext("identb", [128, 128], BF16)
    ext("identf", [128, 128])

    out = nc.dram_tensor("out", [NL, HN], F32, kind="ExternalOutput")
    local_table = nc.dram_tensor("local_table", [NL, 2 * HN], BF16)
    full_table = nc.dram_tensor("full_table", [N, 2 * HN], BF16,
                                addr_space="Shared")
    bn_loc = nc.dram_tensor("bn_loc", [HN, 2], F32)
    bn_sh = nc.dram_tensor("bn_sh", [HN, 2], F32, addr_space="Shared")

    with tile.TileContext(nc) as tc:
        with tc.tile_pool(name="persist", bufs=1) as pp:
            nc.gpsimd.load_library(libcfg.mlp)
            hT = pp.tile([HN, NLP], BF16)
            acc = pp.tile([128, NW, 2 * HN], F32)
            ad_sb = pp.tile([128, NW, HN], BF16)
            w = {}
            for nm, shp, dt in (
                ("fnw", [cfg.IN_N, HN], F32), ("fnb", [HN, 1], F32),
                ("gam", [HN, 1], F32), ("bet", [HN, 1], F32),
                ("fcw", [HN, L, HN], BF16), ("fcb", [HN, L], F32),
                ("wsrc", [HN, L, HN], BF16), ("wdst", [HN, L, HN], BF16),
                ("adb", [HN, L], F32),
                ("u_row", [1, L * HN], BF16),
                ("identb", [128, 128], BF16), ("identf", [128, 128], F32),
                ("dsto", [128, NB * BT], BF16),
                ("iota_mod", [128, BT * 128], BF16),
            ):
                w[nm] = pp.tile(shp, dt, tag=nm, name=nm)
                nc.sync.dma_start(out=w[nm][:], in_=din[nm][:])
            nc.vector.memset(ad_sb[:], 0.0)

            # ---------------- input + BN ----------------
            with tc.tile_pool(name="bn", bufs=2) as bp, \
                 tc.tile_pool(name="bnc", bufs=3) as bq, \
                 tc.tile_pool(name="bnp", bufs=2, space="PSUM") as bpp:
                st = bp.tile([HN, 2], F32, tag="st")
                nc.vector.memset(st[:], 0.0)
                for ci in range(-(-NL // 512)):
                    c0 = ci * 512
                    n = min(512, NL - c0)
                    xt = bq.tile([cfg.IN_N, 512], F32, tag="xt")
                    nc.sync.dma_start(out=xt[:, :n], in_=din["xT"][:, c0:c0 + n])
                    ps = bpp.tile([HN, 512], F32, tag="ps")
                    nc.tensor.matmul(out=ps[:, :n], lhsT=w["fnw"][:],
                                     rhs=xt[:, :n], start=True, stop=True)
                    nc.vector.tensor_scalar_add(hT[:, c0:c0 + n], ps[:, :n],
                                                w["fnb"][:])
                    stc = bq.tile([HN, 2], F32, tag="stc")
                    nc.vector.reduce_sum(stc[:, 0:1], hT[:, c0:c0 + n],
                                         axis=mybir.AxisListType.X)
                    sq = bq.tile([HN, 512], F32, tag="sq")
                    nc.vector.scalar_tensor_tensor(
                        out=sq[:, :n], in0=hT[:, c0:c0 + n], scalar=1.0,
                        in1=hT[:, c0:c0 + n], op0=AX.mult, op1=AX.mult,
                        accum_out=stc[:, 1:2])
                    nc.vector.tensor_add(st[:], st[:], stc[:])
                nc.sync.dma_start(out=bn_loc[:], in_=st[:])
                nc.gpsimd.collective_compute("AllReduce", AX.add,
                                             replica_groups=RG,
                                             ins=[bn_loc[:]], outs=[bn_sh[:]])
                sg = bp.tile([HN, 2], F32, tag="sg")
                nc.sync.dma_start(out=sg[:], in_=bn_sh[:])
                mean = bp.tile([HN, 1], F32, tag="mean")
                var = bp.tile([HN, 1], F32, tag="var")
                nc.vector.tensor_scalar_mul(mean[:], sg[:, 0:1], 1.0 / N)
                nc.vector.tensor_scalar_mul(var[:], sg[:, 1:2], 1.0 / N)
                msq = bp.tile([HN, 1], F32, tag="msq")
                nc.vector.tensor_mul(msq[:], mean[:], mean[:])
                nc.vector.tensor_sub(var[:], var[:], msq[:])
                nc.vector.tensor_scalar_add(var[:], var[:], cfg.EPS)
                rs = bp.tile([HN, 1], F32, tag="rs")
                nc.scalar.activation(out=rs[:], in_=var[:], func=AF.Sqrt)
                nc.vector.reciprocal(rs[:], rs[:])
                scale = bp.tile([HN, 1], F32, tag="scale")
                nc.vector.tensor_mul(scale[:], rs[:], w["gam"][:])
                nbias = bp.tile([HN, 1], F32, tag="nbias")
                nc.vector.tensor_mul(nbias[:], mean[:], scale[:])
                nc.vector.scalar_tensor_tensor(out=nbias[:], in0=nbias[:],
                                               scalar=-1.0, in1=w["bet"][:],
                                               op0=AX.mult, op1=AX.add)
                nc.vector.tensor_scalar(out=hT[:, :NL], in0=hT[:, :NL],
                                        scalar1=scale[:], scalar2=nbias[:],
                                        op0=AX.mult, op1=AX.add)

            # ---------------- layers (interleaved emission) ----------------
            # last-touch block for each window; finalize/table emitted there
            lt = {}
            for b in range(NB):
                for t in range(int(nlive[b])):
                    lt[int(wsched[b, t])] = b
            fin_after = {b: [] for b in range(NB)}
            for wv in range(NW):
                fin_after[lt.get(wv, NB - 1)].append(wv)

            with tc.tile_pool(name="tb", bufs=3) as tp, \
                 tc.tile_pool(name="tq", bufs=1, space="PSUM") as tqq, \
                 tc.tile_pool(name="tp2", bufs=1, space="PSUM") as tpp, \
                 tc.tile_pool(name="ep", bufs=2) as ep, \
                 tc.tile_pool(name="pp", bufs=2, space="PSUM") as epp, \
                 tc.tile_pool(name="wp", bufs=3, space="PSUM") as wpp:

                def table_build(l, wv):
                    # hT cols [wv*128, +n) -> local_table rows + ad_sb window
                    c0 = wv * 128
                    n = min(128, NL - c0)
                    psh = tpp.tile([HN, 128], F32, tag="psh")
                    nc.tensor.matmul(out=psh[:, :n], lhsT=w["fcw"][:, l, :],
                                     rhs=hT[:, c0:c0 + n], start=True, stop=True)
                    hl = tp.tile([HN, 128], BF16, tag="hl")
                    nc.vector.tensor_scalar_add(hl[:, :n], psh[:, :n],
                                                w["fcb"][:, l:l + 1])
                    pss = tpp.tile([HN, 128], F32, tag="psh")
                    nc.tensor.matmul(out=pss[:, :n], lhsT=w["wsrc"][:, l, :],
                                     rhs=hl[:, :n], start=True, stop=True)
                    asb = tp.tile([HN, 128], BF16, tag="asb")
                    nc.scalar.copy(out=asb[:, :n], in_=pss[:, :n])
                    psd = tpp.tile([HN, 128], F32, tag="psh")
                    nc.tensor.matmul(out=psd[:, :n], lhsT=w["wdst"][:, l, :],
                                     rhs=hl[:, :n], start=True, stop=True)
                    adt = tp.tile([HN, 128], BF16, tag="adt")
                    nc.vector.tensor_scalar_add(adt[:, :n], psd[:, :n],
                                                w["adb"][:, l:l + 1])
                    rows = tp.tile([128, 2 * HN], BF16, tag="rows")
                    pt = tpp.tile([128, HN], BF16, tag="pt")
                    nc.tensor.transpose(out=pt[:n, :], in_=asb[:, :n],
                                        identity=w["identb"][:HN, :HN])
                    nc.scalar.copy(out=rows[:n, 0:HN], in_=pt[:n, :])
                    pt2 = tpp.tile([128, HN], BF16, tag="pt")
                    nc.tensor.transpose(out=pt2[:n, :], in_=hl[:, :n],
                                        identity=w["identb"][:HN, :HN])
                    nc.scalar.copy(out=rows[:n, HN:2 * HN], in_=pt2[:n, :])
                    nc.sync.dma_start(out=local_table[c0:c0 + n, :],
                                      in_=rows[:n, :])
                    pt3 = tpp.tile([128, HN], BF16, tag="pt")
                    nc.tensor.transpose(out=pt3[:n, :], in_=adt[:, :n],
                                        identity=w["identb"][:HN, :HN])
                    nc.vector.tensor_copy(ad_sb[:n, wv, :], pt3[:n, :])

                def finalize_hT(wv):
                    ptr = tqq.tile([128, 128], F32, tag="ptr")
                    nc.tensor.transpose(out=ptr[:], in_=acc[:, wv, :],
                                        identity=w["identf"][:])
                    den = tp.tile([HN, 128], F32, tag="den")
                    nc.vector.tensor_scalar_add(den[:], ptr[HN:2 * HN, :], 1e-16)
                    nc.vector.reciprocal(den[:], den[:])
                    nc.vector.tensor_mul(hT[:, wv * 128:(wv + 1) * 128],
                                         ptr[0:HN, :], den[:])

                def output_rows(wv):
                    n = min(128, NL - wv * 128)
                    den = tp.tile([128, HN], F32, tag="den2")
                    nc.vector.tensor_scalar_add(
                        den[:n, :], acc[:n, wv, HN:2 * HN], 1e-16)
                    nc.vector.reciprocal(den[:n, :], den[:n, :])
                    orow = tp.tile([128, HN], F32, tag="orow")
                    nc.vector.tensor_mul(orow[:n, :], acc[:n, wv, 0:HN],
                                         den[:n, :])
                    nc.sync.dma_start(out=out[wv * 128:wv * 128 + n, :],
                                      in_=orow[:n, :])

                def emit_block(l, b):
                    g = int(gob[b])
                    nl_b = int(nlive[b])
                    gix = ep.tile([128, EB // 16], I16, tag="gix")
                    nc.sync.dma_start(
                        out=gix[:],
                        in_=din["gidx"][:, b * (EB // 16):(b + 1) * (EB // 16)])
                    mtw_t = ep.tile([128, EB], F8, tag="mtw_t")
                    nc.sync.dma_start(out=mtw_t[:], in_=din["mtw"][b, :, :])
                    eat = ep.tile([1, EB], BF16, tag="eat", bufs=1)
                    nc.sync.dma_start(out=eat[:], in_=din["eab"][b:b + 1, :])
                    mt_ew = ep.tile([128, BT, 128], BF16, tag="mt_ew")
                    nc.vector.tensor_tensor(
                        out=mt_ew[:],
                        in0=w["dsto"][:, b * BT:(b + 1) * BT].rearrange(
                            "p (t o) -> p t o", o=1).to_broadcast([128, BT, 128]),
                        in1=w["iota_mod"][:].rearrange("p (t s) -> p t s", s=128),
                        op=AX.is_equal)
                    srcr = ep.tile([128, BT, 2 * HN], BF16, tag="srcr")
                    for hh in range(4):
                        live = min(EB // 4, max(0, nl_b * TE - hh * (EB // 4)))
                        if live == 0:
                            continue
                        nc.gpsimd.dma_gather(
                            out_ap=srcr[:, hh * (BT // 4):(hh + 1) * (BT // 4), :],
                            in_ap=full_table[g * CHUNK:(g + 1) * CHUNK, :],
                            idxs_ap=gix[:, hh * (EB // 64):(hh + 1) * (EB // 64)],
                            num_idxs=EB // 4, num_idxs_reg=live,
                            elem_size=2 * HN,
                            single_packet=True, queue_num=(b + hh) % 4)
                    alpha = ep.tile([128, BT, HN], F32, tag="alpha")
                    contrib = ep.tile([128, BT, 2 * HN], BF16, tag="contrib")
                    for grp in range(-(-nl_b // 8)):
                        t0 = grp * 8
                        gn = min(8, nl_b - t0)
                        psx = epp.tile([128, 8 * HN], F32, tag="psx")
                        for t2 in range(gn):
                            t = t0 + t2
                            nc.tensor.matmul(
                                out=psx[:, t2 * HN:(t2 + 1) * HN],
                                lhsT=mtw_t[:, t * TE:(t + 1) * TE],
                                rhs=ad_sb[:, int(wsched[b, t]), :],
                                start=True, stop=False)
                            nc.tensor.matmul(
                                out=psx[:, t2 * HN:(t2 + 1) * HN],
                                lhsT=eat[0:1, t * TE:(t + 1) * TE],
                                rhs=w["u_row"][:, l * HN:(l + 1) * HN],
                                start=False, stop=True)
                        nc.vector.tensor_tensor(
                            out=alpha[:, t0:t0 + gn, :],
                            in0=psx[:, :gn * HN].rearrange(
                                "p (t h) -> p t h", h=HN),
                            in1=srcr[:, t0:t0 + gn, 0:HN],
                            op=AX.add)
                    extmp = ep.tile([128, BT, HN], BF16, tag="extmp")
                    nc.scalar.activation(out=contrib[:, :nl_b, HN:2 * HN],
                                         in_=alpha[:, :nl_b, :], func=AF.Exp)
                    nc.scalar.activation(out=extmp[:, :nl_b, :],
                                         in_=alpha[:, :nl_b, :], func=AF.Exp,
                                         scale=cfg.NEG)
                    nc.vector.tensor_tensor(
                        out=contrib[:, :nl_b, HN:2 * HN],
                        in0=contrib[:, :nl_b, HN:2 * HN],
                        in1=extmp[:, :nl_b, :], op=AX.max)
                    nc.vector.tensor_mul(contrib[:, :nl_b, 0:HN],
                                         contrib[:, :nl_b, HN:2 * HN],
                                         srcr[:, :nl_b, HN:2 * HN])
                    t = 0
                    while t < nl_b:
                        wv = int(wsched[b, t])
                        t1 = t
                        while t1 + 1 < nl_b and wsched[b, t1 + 1] == wv:
                            t1 += 1
                        wps = wpp.tile([128, 2 * HN], F32, tag="wps")
                        for tt in range(t, t1 + 1):
                            nc.tensor.matmul(
                                out=wps[:], lhsT=mt_ew[:, tt, :],
                                rhs=contrib[:, tt, :],
                                start=(tt == t), stop=(tt == t1))
                        nc.vector.tensor_add(acc[:, wv, :], acc[:, wv, :],
                                             wps[:])
                        t = t1 + 1

                # layer 0 table from BN hT + acc init
                for wv in range(NW):
                    table_build(0, wv)
                    nc.vector.memset(acc[:, wv, :], 0.0)
                nc.gpsimd.collective_compute("AllGather", AX.bypass,
                                             replica_groups=RG,
                                             ins=[local_table[:]],
                                             outs=[full_table[:]])
                for l in range(L):
                    for b in range(NB):
                        emit_block(l, b)
                        for wv in fin_after[b]:
                            if l < L - 1:
                                finalize_hT(wv)
                                table_build(l + 1, wv)
                                nc.vector.memset(acc[:, wv, :], 0.0)
                            else:
                                output_rows(wv)
                    if l < L - 1:
                        nc.gpsimd.collective_compute(
                            "AllGather", AX.bypass, replica_groups=RG,
                            ins=[local_table[:]], outs=[full_table[:]])
    return nc


# --------------------------------------------------------------- entry ------
def kernel(**inputs):
    cfg = Cfg()
    in_maps, wsched, gob, nbg = host_prepare(cfg, inputs)
    nc = build_nc(cfg, wsched, gob, nbg)
    nc.compile()
    split_sync_waits(nc)
    res = run_bass_kernel_spmd(nc, in_maps, core_ids=list(range(cfg.C)),
                               trace=bool(int(os.environ.get("GNN_TRACE", "0"))))
    if res.exec_time_ns is not None:
        print(f"HW exec time: {res.exec_time_ns} ns")
    if res.instructions_and_trace is not None:
        print(f"trace: {res.instructions_and_trace[1]}")
    h = np.concatenate([np.asarray(res.results[c]["out"])
                        for c in range(cfg.C)], axis=0)
    return h.reshape(cfg.B, -1, cfg.HN)


# revision 18
# speedup vs baseline: 1.0283x; 1.0283x over previous
"""GNN message-passing encoder (GAT-style) on 8 Trainium2 NeuronCores.

Self-contained: hardcodes the problem shapes (N=100000, E=1600000, HN=64, L=3).

Sharding: core c owns dst nodes [c*NL, (c+1)*NL). Edges sorted by
(src_chunk g, dst); dst space split into 128-aligned windows; per (g, window)
the tile count T_gw is the max over cores (shared compile-time schedule).

Per layer, on device (all bf16 tables):
- table build: hlT = fc_w[l]^T h; a_s = W_src hl; a_d = W_dst hl + bias;
  node rows [a_s|hl] -> local_table -> AllGather full_table (256B rows);
  a_d rows transposed into SBUF-resident ad_sb[128, NW, 64].
- per block (4096 edges): dma_gather 4096 src rows (bf16, 1 desc each);
  host-built one-hot mtw [128 dstoff x 4096 e] from HBM; expand matmul
  psx = mtw^T @ ad_sb[:, w, :] per tile -> per-edge a_d; alpha = psx + a_s
  + ea*u (broadcast DVE); LRelu; Exp -> contrib [ex*hl | ex] bf16;
  aggregation matmul per tile accumulates into per-window PSUM
  (mt_ew one-hot built on-device); window flush adds into SBUF acc (f32).
- finalize from SBUF acc: h = num/(den+1e-16) -> hT bf16 (or out rows f32).
No HBM scatter/accumulators; only per-edge gather remains indexed.
"""
import os
import sys

for _p in ("/opt/trn_rl_repo",):
    if _p not in sys.path:
        sys.path.insert(0, _p)

import numpy as np
import ml_dtypes

import concourse.bass as bass
import concourse.mybir as mybir
import concourse.tile as tile
from concourse import bacc
from concourse import library_config as libcfg
from concourse.bass_utils import run_bass_kernel_spmd

F32 = mybir.dt.float32
BF16 = mybir.dt.bfloat16
I16 = mybir.dt.int16
AX = mybir.AluOpType
AF = mybir.ActivationFunctionType
BF16NP = ml_dtypes.bfloat16


class Cfg:
    def __init__(self):
        self.N, self.E, self.C = 100000, 1600000, 8
        self.HN, self.IN_N, self.L, self.B = 64, 3, 3, 10
        self.NEG, self.EPS = 0.2, 1e-5
        self.NL = self.N // self.C          # 12500
        self.CHUNK = 25000
        self.G = self.N // self.CHUNK       # 4
        self.TE = 128
        self.BT = 32
        self.EB = self.TE * self.BT         # 4096
        self.NW = -(-self.NL // 128)        # 98
        self.NLP = self.NW * 128            # 12544


# ------------------------------------------------------------- host prep ----
def _pack_idx16(vals):
    v = np.asarray(vals, dtype=np.int16)
    assert v.size % 16 == 0
    return np.tile(v.reshape(-1, 16).T, (8, 1)).copy()


def host_prepare(cfg, inputs):
    C, G, NL, CHUNK, NW = cfg.C, cfg.G, cfg.NL, cfg.CHUNK, cfg.NW
    TE, BT, EB, HN, L = cfg.TE, cfg.BT, cfg.EB, cfg.HN, cfg.L

    ei = np.asarray(inputs["edge_index"]).astype(np.int64)
    src, dst = ei[0], ei[1]
    ea_all = np.asarray(inputs["edge_attr"], np.float32)[:, 0]
    core = dst // NL
    dstl = dst - core * NL
    gch = src // CHUNK
    win = dstl >> 7
    dof = dstl & 127

    # shared tile schedule: T_gw = max over cores of ceil(cnt/128)
    key = (core * G + gch) * NW + win
    cnt = np.bincount(key, minlength=C * G * NW).reshape(C, G, NW)
    Tgw = -(-cnt.max(axis=0) // TE)                    # [G, NW]
    tiles_g = Tgw.sum(axis=1)                          # [G]
    nbg = np.maximum(1, -(-tiles_g // BT))
    NB = int(nbg.sum())
    tile_start = np.cumsum(Tgw, axis=1) - Tgw          # [G, NW]
    block_base = np.concatenate([[0], np.cumsum(nbg)])[:G]

    # per-block window schedule (window id per tile slot, -1 = padding)
    wsched = -np.ones((NB, BT), np.int64)
    for g in range(G):
        flat = np.repeat(np.arange(NW), Tgw[g])
        b = block_base[g] + np.arange(len(flat)) // BT
        wsched[b, np.arange(len(flat)) % BT] = flat
    gob = np.repeat(np.arange(G), nbg)

    # weights
    fnw = np.asarray(inputs["fc_node_w"], np.float32)
    fnb = np.asarray(inputs["fc_node_b"], np.float32)
    few = np.asarray(inputs["fc_edge_w"], np.float32)
    feb = np.asarray(inputs["fc_edge_b"], np.float32)
    gam = np.asarray(inputs["bn_gamma"], np.float32)
    bet = np.asarray(inputs["bn_beta"], np.float32)
    fcw = np.asarray(inputs["fc_w"], np.float32)
    fcb = np.asarray(inputs["fc_b"], np.float32)
    aw = np.asarray(inputs["attn_w"], np.float32)
    ab = np.asarray(inputs["attn_b"], np.float32)
    wdst, wsrc, we = aw[:, :HN, :], aw[:, HN:2 * HN, :], aw[:, 2 * HN:, :]
    u = np.stack([few[0] @ we[l] for l in range(L)])          # [L, 64]
    adb = np.stack([ab[l] + feb @ we[l] for l in range(L)])   # [L, 64]
    x = np.asarray(inputs["x"], np.float32)

    shared = dict(
        fnw=fnw, fnb=fnb.reshape(-1, 1), gam=gam.reshape(-1, 1),
        bet=bet.reshape(-1, 1),
        fcw=np.ascontiguousarray(fcw.transpose(1, 0, 2)).astype(BF16NP),
        wsrc=np.ascontiguousarray(wsrc.transpose(1, 0, 2)).astype(BF16NP),
        wdst=np.ascontiguousarray(wdst.transpose(1, 0, 2)).astype(BF16NP),
        fcb=np.ascontiguousarray(fcb.T), adb=np.ascontiguousarray(adb.T),
        u_row=u.reshape(1, -1).astype(BF16NP),
        iota_mod=np.tile(np.arange(128, dtype=BF16NP), (128, BT)),
        identb=np.eye(128, dtype=BF16NP),
        identf=np.eye(128, dtype=np.float32),
    )

    in_maps = []
    for c in range(C):
        m = core == c
        sc, gc, wc, dfc, eac, dlc = (src[m], gch[m], win[m], dof[m],
                                     ea_all[m], dstl[m])
        order = np.lexsort((dlc, gc))
        sc, gc, wc, dfc, eac = (sc[order], gc[order], wc[order], dfc[order],
                                eac[order])
        gw = gc * NW + wc
        cnts = np.bincount(gw, minlength=G * NW)
        starts = np.cumsum(cnts) - cnts
        k = np.arange(sc.size) - starts[gw]
        tin = k >> 7
        p = k & 127
        tgi = tile_start[gc, wc] + tin
        blk = block_base[gc] + tgi // BT
        tib = tgi % BT
        slot = blk * EB + tib * TE + p

        gidx = np.zeros(NB * EB, np.int16)
        gidx[slot] = (sc - gc * CHUNK).astype(np.int16)
        mtw = np.zeros((NB, 128, EB), BF16NP)
        mtw[blk, dfc, tib * TE + p] = 1.0
        dsto = np.full((128, NB * BT), 255, BF16NP)
        dsto[p, blk * BT + tib] = dfc.astype(BF16NP)
        eab = np.zeros((NB, EB), BF16NP)
        eab[blk, tib * TE + p] = eac.astype(BF16NP)

        im = dict(shared)
        im.update(gidx=_pack_idx16(gidx), mtw=mtw, dsto=dsto, eab=eab,
                  xT=np.ascontiguousarray(x[c * NL:(c + 1) * NL].T))
        in_maps.append(im)
    return in_maps, wsched, gob, nbg


# --------------------------------------------------------------- builder ----
def split_sync_waits(nc, max_waits=1):
    for f in nc.m.functions:
        for bb in f.blocks:
            old = bb.instructions
            if not any(i.sync_info and i.sync_info.on_wait
                       and len(i.sync_info.on_wait) > max_waits for i in old):
                continue
            new = []
            for ins in old:
                si = ins.sync_info
                if si is not None and si.on_wait and len(si.on_wait) > max_waits:
                    waits = list(si.on_wait)
                    extra, keep = waits[:-max_waits], waits[-max_waits:]
                    for j, w in enumerate(extra):
                        nop = mybir.InstNoOp(name=f"{ins.name}-wc{j}", ins=[], outs=[])
                        nop.engine = ins.engine
                        nop.sync_info = mybir.SyncInfo(on_wait=[w], on_update=[])
                        new.append(nop)
                    si.on_wait = keep
                new.append(ins)
            bb.instructions = new
    return nc


def build_nc(cfg, wsched, gob, nbg):
    NL, G, CHUNK, TE, BT, EB = cfg.NL, cfg.G, cfg.CHUNK, cfg.TE, cfg.BT, cfg.EB
    HN, L, C, N, NW, NLP = cfg.HN, cfg.L, cfg.C, cfg.N, cfg.NW, cfg.NLP
    NB = int(sum(nbg))
    RG = [list(range(C))]
    # live tile count per block (live tiles form a prefix)
    nlive = [(wsched[b] >= 0).sum() for b in range(NB)]

    nc = bacc.Bacc(None, target_bir_lowering=False, num_swdge_queues=4)
    din = {}

    def ext(name, shape, dt=F32):
        din[name] = nc.dram_tensor(name, shape, dt, kind="ExternalInput")

    ext("xT", [cfg.IN_N, NL])
    ext("gidx", [128, NB * EB // 16], I16)
    ext("mtw", [NB, 128, EB], BF16)
    ext("dsto", [128, NB * BT], BF16)
    ext("eab", [NB, EB], BF16)
    ext("iota_mod", [128, BT * 128], BF16)
    ext("fnw", [cfg.IN_N, HN]); ext("fnb", [HN, 1])
    ext("gam", [HN, 1]); ext("bet", [HN, 1])
    ext("fcw", [HN, L, HN], BF16); ext("fcb", [HN, L])
    ext("wsrc", [HN, L, HN], BF16); ext("wdst", [HN, L, HN], BF16)
    ext("adb", [HN, L])
    ext("u_row", [1, L * HN], BF16)
    ext("identb", [128, 128], BF16)
    ext("identf", [128, 128])

    out = nc.dram_tensor("out", [NL, HN], F32, kind="ExternalOutput")
    local_table = nc.dram_tensor("local_table", [NL, 2 * HN], BF16)
    full_table = nc.dram_tensor("full_table", [N, 2 * HN], BF16,
                                addr_space="Shared")
    bn_loc = nc.dram_tensor("bn_loc", [HN, 2], F32)
    bn_sh = nc.dram_tensor("bn_sh", [HN, 2], F32, addr_space="Shared")

    with tile.TileContext(nc) as tc:
        with tc.tile_pool(name="persist", bufs=1) as pp:
            nc.gpsimd.load_library(libcfg.mlp)
            hT = pp.tile([HN, NLP], BF16)
            acc = pp.tile([128, NW, 2 * HN], F32)
            ad_sb = pp.tile([128, NW, HN], BF16)
            w = {}
            for nm, shp, dt in (
                ("fnw", [cfg.IN_N, HN], F32), ("fnb", [HN, 1], F32),
                ("gam", [HN, 1], F32), ("bet", [HN, 1], F32),
                ("fcw", [HN, L, HN], BF16), ("fcb", [HN, L], F32),
                ("wsrc", [HN, L, HN], BF16), ("wdst", [HN, L, HN], BF16),
                ("adb", [HN, L], F32),
                ("u_row", [1, L * HN], BF16),
                ("identb", [128, 128], BF16), ("identf", [128, 128], F32),
                ("dsto", [128, NB * BT], BF16),
                ("iota_mod", [128, BT * 128], BF16),
            ):
                w[nm] = pp.tile(shp, dt, tag=nm, name=nm)
                nc.sync.dma_start(out=w[nm][:], in_=din[nm][:])
            nc.vector.memset(ad_sb[:], 0.0)

            # ---------------- input + BN ----------------
            with tc.tile_pool(name="bn", bufs=2) as bp, \
                 tc.tile_pool(name="bnc", bufs=3) as bq, \
                 tc.tile_pool(name="bnp", bufs=2, space="PSUM") as bpp:
                st = bp.tile([HN, 2], F32, tag="st")
                nc.vector.memset(st[:], 0.0)
                for ci in range(-(-NL // 512)):
                    c0 = ci * 512
                    n = min(512, NL - c0)
                    xt = bq.tile([cfg.IN_N, 512], F32, tag="xt")
                    nc.sync.dma_start(out=xt[:, :n], in_=din["xT"][:, c0:c0 + n])
                    ps = bpp.tile([HN, 512], F32, tag="ps")
                    nc.tensor.matmul(out=ps[:, :n], lhsT=w["fnw"][:],
                                     rhs=xt[:, :n], start=True, stop=True)
                    nc.vector.tensor_scalar_add(hT[:, c0:c0 + n], ps[:, :n],
                                                w["fnb"][:])
                    stc = bq.tile([HN, 2], F32, tag="stc")
                    nc.vector.reduce_sum(stc[:, 0:1], hT[:, c0:c0 + n],
                                         axis=mybir.AxisListType.X)
                    sq = bq.tile([HN, 512], F32, tag="sq")
                    nc.vector.scalar_tensor_tensor(
                        out=sq[:, :n], in0=hT[:, c0:c0 + n], scalar=1.0,
                        in1=hT[:, c0:c0 + n], op0=AX.mult, op1=AX.mult,
                        accum_out=stc[:, 1:2])
                    nc.vector.tensor_add(st[:], st[:], stc[:])
                nc.sync.dma_start(out=bn_loc[:], in_=st[:])
                nc.gpsimd.collective_compute("AllReduce", AX.add,
                                             replica_groups=RG,
                                             ins=[bn_loc[:]], outs=[bn_sh[:]])
                sg = bp.tile([HN, 2], F32, tag="sg")
                nc.sync.dma_start(out=sg[:], in_=bn_sh[:])
                mean = bp.tile([HN, 1], F32, tag="mean")
                var = bp.tile([HN, 1], F32, tag="var")
                nc.vector.tensor_scalar_mul(mean[:], sg[:, 0:1], 1.0 / N)
                nc.vector.tensor_scalar_mul(var[:], sg[:, 1:2], 1.0 / N)
                msq = bp.tile([HN, 1], F32, tag="msq")
                nc.vector.tensor_mul(msq[:], mean[:], mean[:])
                nc.vector.tensor_sub(var[:], var[:], msq[:])
                nc.vector.tensor_scalar_add(var[:], var[:], cfg.EPS)
                rs = bp.tile([HN, 1], F32, tag="rs")
                nc.scalar.activation(out=rs[:], in_=var[:], func=AF.Sqrt)
                nc.vector.reciprocal(rs[:], rs[:])
                scale = bp.tile([HN, 1], F32, tag="scale")
                nc.vector.tensor_mul(scale[:], rs[:], w["gam"][:])
                nbias = bp.tile([HN, 1], F32, tag="nbias")
                nc.vector.tensor_mul(nbias[:], mean[:], scale[:])
                nc.vector.scalar_tensor_tensor(out=nbias[:], in0=nbias[:],
                                               scalar=-1.0, in1=w["bet"][:],
                                               op0=AX.mult, op1=AX.add)
                nc.vector.tensor_scalar(out=hT[:, :NL], in0=hT[:, :NL],
                                        scalar1=scale[:], scalar2=nbias[:],
                                        op0=AX.mult, op1=AX.add)

            # ---------------- layers (interleaved emission) ----------------
            # last-touch block for each window; finalize/table emitted there
            lt = {}
            for b in range(NB):
                for t in range(int(nlive[b])):
                    lt[int(wsched[b, t])] = b
            fin_after = {b: [] for b in range(NB)}
            for wv in range(NW):
                fin_after[lt.get(wv, NB - 1)].append(wv)

            with tc.tile_pool(name="tb", bufs=3) as tp, \
                 tc.tile_pool(name="tq", bufs=1, space="PSUM") as tqq, \
                 tc.tile_pool(name="tp2", bufs=1, space="PSUM") as tpp, \
                 tc.tile_pool(name="ep", bufs=2) as ep, \
                 tc.tile_pool(name="pp", bufs=2, space="PSUM") as epp, \
                 tc.tile_pool(name="wp", bufs=3, space="PSUM") as wpp:

                def table_build(l, wv):
                    # hT cols [wv*128, +n) -> local_table rows + ad_sb window
                    c0 = wv * 128
                    n = min(128, NL - c0)
                    psh = tpp.tile([HN, 128], F32, tag="psh")
                    nc.tensor.matmul(out=psh[:, :n], lhsT=w["fcw"][:, l, :],
                                     rhs=hT[:, c0:c0 + n], start=True, stop=True)
                    hl = tp.tile([HN, 128], BF16, tag="hl")
                    nc.vector.tensor_scalar_add(hl[:, :n], psh[:, :n],
                                                w["fcb"][:, l:l + 1])
                    pss = tpp.tile([HN, 128], F32, tag="psh")
                    nc.tensor.matmul(out=pss[:, :n], lhsT=w["wsrc"][:, l, :],
                                     rhs=hl[:, :n], start=True, stop=True)
                    asb = tp.tile([HN, 128], BF16, tag="asb")
                    nc.scalar.copy(out=asb[:, :n], in_=pss[:, :n])
                    psd = tpp.tile([HN, 128], F32, tag="psh")
                    nc.tensor.matmul(out=psd[:, :n], lhsT=w["wdst"][:, l, :],
                                     rhs=hl[:, :n], start=True, stop=True)
                    adt = tp.tile([HN, 128], BF16, tag="adt")
                    nc.vector.tensor_scalar_add(adt[:, :n], psd[:, :n],
                                                w["adb"][:, l:l + 1])
                    rows = tp.tile([128, 2 * HN], BF16, tag="rows")
                    pt = tpp.tile([128, HN], BF16, tag="pt")
                    nc.tensor.transpose(out=pt[:n, :], in_=asb[:, :n],
                                        identity=w["identb"][:HN, :HN])
                    nc.scalar.copy(out=rows[:n, 0:HN], in_=pt[:n, :])
                    pt2 = tpp.tile([128, HN], BF16, tag="pt")
                    nc.tensor.transpose(out=pt2[:n, :], in_=hl[:, :n],
                                        identity=w["identb"][:HN, :HN])
                    nc.scalar.copy(out=rows[:n, HN:2 * HN], in_=pt2[:n, :])
                    nc.sync.dma_start(out=local_table[c0:c0 + n, :],
                                      in_=rows[:n, :])
                    pt3 = tpp.tile([128, HN], BF16, tag="pt")
                    nc.tensor.transpose(out=pt3[:n, :], in_=adt[:, :n],
                                        identity=w["identb"][:HN, :HN])
                    nc.vector.tensor_copy(ad_sb[:n, wv, :], pt3[:n, :])

                def finalize_hT(wv):
                    ptr = tqq.tile([128, 128], F32, tag="ptr")
                    nc.tensor.transpose(out=ptr[:], in_=acc[:, wv, :],
                                        identity=w["identf"][:])
                    den = tp.tile([HN, 128], F32, tag="den")
                    nc.vector.tensor_scalar_add(den[:], ptr[HN:2 * HN, :], 1e-16)
                    nc.vector.reciprocal(den[:], den[:])
                    nc.vector.tensor_mul(hT[:, wv * 128:(wv + 1) * 128],
                                         ptr[0:HN, :], den[:])

                def output_rows(wv):
                    n = min(128, NL - wv * 128)
                    den = tp.tile([128, HN], F32, tag="den2")
                    nc.vector.tensor_scalar_add(
                        den[:n, :], acc[:n, wv, HN:2 * HN], 1e-16)
                    nc.vector.reciprocal(den[:n, :], den[:n, :])
                    orow = tp.tile([128, HN], F32, tag="orow")
                    nc.vector.tensor_mul(orow[:n, :], acc[:n, wv, 0:HN],
                                         den[:n, :])
                    nc.sync.dma_start(out=out[wv * 128:wv * 128 + n, :],
                                      in_=orow[:n, :])

                def emit_block(l, b):
                    g = int(gob[b])
                    nl_b = int(nlive[b])
                    gix = ep.tile([128, EB // 16], I16, tag="gix")
                    nc.sync.dma_start(
                        out=gix[:],
                        in_=din["gidx"][:, b * (EB // 16):(b + 1) * (EB // 16)])
                    mtw_t = ep.tile([128, EB], BF16, tag="mtw_t")
                    nc.sync.dma_start(out=mtw_t[:], in_=din["mtw"][b, :, :])
                    eat = ep.tile([1, EB], BF16, tag="eat", bufs=1)
                    nc.sync.dma_start(out=eat[:], in_=din["eab"][b:b + 1, :])
                    mt_ew = ep.tile([128, BT, 128], BF16, tag="mt_ew")
                    nc.vector.tensor_tensor(
                        out=mt_ew[:],
                        in0=w["dsto"][:, b * BT:(b + 1) * BT].rearrange(
                            "p (t o) -> p t o", o=1).to_broadcast([128, BT, 128]),
                        in1=w["iota_mod"][:].rearrange("p (t s) -> p t s", s=128),
                        op=AX.is_equal)
                    srcr_q = [ep.tile([128, BT // 4, 2 * HN], BF16,
                                      tag=f"srcr{hh}", name=f"srcr{hh}")
                              for hh in range(4)]
                    for hh in range(4):
                        live = min(EB // 4, max(0, nl_b * TE - hh * (EB // 4)))
                        if live == 0:
                            continue
                        nc.gpsimd.dma_gather(
                            out_ap=srcr_q[hh][:],
                            in_ap=full_table[g * CHUNK:(g + 1) * CHUNK, :],
                            idxs_ap=gix[:, hh * (EB // 64):(hh + 1) * (EB // 64)],
                            num_idxs=EB // 4, num_idxs_reg=live,
                            elem_size=2 * HN,
                            single_packet=True, queue_num=(b + hh) % 4)
                    alpha = ep.tile([128, BT, HN], F32, tag="alpha")
                    contrib = ep.tile([128, BT, 2 * HN], BF16, tag="contrib")
                    for grp in range(-(-nl_b // 8)):
                        t0 = grp * 8
                        gn = min(8, nl_b - t0)
                        psx = epp.tile([128, 8 * HN], F32, tag="psx")
                        for t2 in range(gn):
                            t = t0 + t2
                            nc.tensor.matmul(
                                out=psx[:, t2 * HN:(t2 + 1) * HN],
                                lhsT=mtw_t[:, t * TE:(t + 1) * TE],
                                rhs=ad_sb[:, int(wsched[b, t]), :],
                                start=True, stop=False)
                            nc.tensor.matmul(
                                out=psx[:, t2 * HN:(t2 + 1) * HN],
                                lhsT=eat[0:1, t * TE:(t + 1) * TE],
                                rhs=w["u_row"][:, l * HN:(l + 1) * HN],
                                start=False, stop=True)
                        nc.vector.tensor_tensor(
                            out=alpha[:, t0:t0 + gn, :],
                            in0=psx[:, :gn * HN].rearrange(
                                "p (t h) -> p t h", h=HN),
                            in1=srcr_q[grp][:, 0:gn, 0:HN],
                            op=AX.add)
                    extmp = ep.tile([128, BT, HN], BF16, tag="extmp")
                    nc.scalar.activation(out=contrib[:, :nl_b, HN:2 * HN],
                                         in_=alpha[:, :nl_b, :], func=AF.Exp)
                    nc.scalar.activation(out=extmp[:, :nl_b, :],
                                         in_=alpha[:, :nl_b, :], func=AF.Exp,
                                         scale=cfg.NEG)
                    nc.vector.tensor_tensor(
                        out=contrib[:, :nl_b, HN:2 * HN],
                        in0=contrib[:, :nl_b, HN:2 * HN],
                        in1=extmp[:, :nl_b, :], op=AX.max)
                    for grp in range(-(-nl_b // 8)):
                        t0 = grp * 8
                        gn = min(8, nl_b - t0)
                        nc.vector.tensor_mul(contrib[:, t0:t0 + gn, 0:HN],
                                             contrib[:, t0:t0 + gn, HN:2 * HN],
                                             srcr_q[grp][:, 0:gn, HN:2 * HN])
                    t = 0
                    while t < nl_b:
                        wv = int(wsched[b, t])
                        t1 = t
                        while t1 + 1 < nl_b and wsched[b, t1 + 1] == wv:
                            t1 += 1
                        wps = wpp.tile([128, 2 * HN], F32, tag="wps")
                        for tt in range(t, t1 + 1):
                            nc.tensor.matmul(
                                out=wps[:], lhsT=mt_ew[:, tt, :],
                                rhs=contrib[:, tt, :],
                                start=(tt == t), stop=(tt == t1))
                        nc.vector.tensor_add(acc[:, wv, :], acc[:, wv, :],
                                             wps[:])
                        t = t1 + 1

                # layer 0 table from BN hT + acc init
                for wv in range(NW):
                    table_build(0, wv)
                    nc.vector.memset(acc[:, wv, :], 0.0)
                nc.gpsimd.collective_compute("AllGather", AX.bypass,
                                             replica_groups=RG,
                                             ins=[local_table[:]],
                                             outs=[full_table[:]])
                for l in range(L):
                    for b in range(NB):
                        emit_block(l, b)
                        for wv in fin_after[b]:
                            if l < L - 1:
                                finalize_hT(wv)
                                table_build(l + 1, wv)
                                nc.vector.memset(acc[:, wv, :], 0.0)
                            else:
                                output_rows(wv)
                    if l < L - 1:
                        nc.gpsimd.collective_compute(
                            "AllGather", AX.bypass, replica_groups=RG,
                            ins=[local_table[:]], outs=[full_table[:]])
    return nc


# --------------------------------------------------------------- entry ------
def kernel(**inputs):
    cfg = Cfg()
    in_maps, wsched, gob, nbg = host_prepare(cfg, inputs)
    nc = build_nc(cfg, wsched, gob, nbg)
    nc.compile()
    split_sync_waits(nc)
    res = run_bass_kernel_spmd(nc, in_maps, core_ids=list(range(cfg.C)),
                               trace=bool(int(os.environ.get("GNN_TRACE", "0"))))
    if res.exec_time_ns is not None:
        print(f"HW exec time: {res.exec_time_ns} ns")
    if res.instructions_and_trace is not None:
        print(f"trace: {res.instructions_and_trace[1]}")
    h = np.concatenate([np.asarray(res.results[c]["out"])
                        for c in range(cfg.C)], axis=0)
    return h.reshape(cfg.B, -1, cfg.HN)
